# revision 54
# baseline (speedup 1.0000x reference)
"""MoE feed-forward (top-1 routing) on 8 TRN2 NeuronCores.

Sharding: expert-parallel with a tensor-parallel split of the hidden dim,
load-balanced by pairing experts.  Each expert's H=3072 hidden is split in
4 quarters (HQ=768).  Sorting experts by routed-token count (r0 >= r1 >=
r2 >= r3), slot A of core c holds a quarter of r0 (cores 0-3) or r1 (4-7),
slot B holds a quarter of r2 / r3.  Every core runs, per slot,

    y_q = GELU(x_e @ W1[e][:, q] + b1[e][q]) @ W2[e][q, :]

over that slot's capacity (C1 = padded max(r0,r1) count, C2 = padded
max(r2,r3)), and the host sums the 4 quarter-partials per expert, adds b2,
and scatters back.  Pairing a heavy and a light expert per core makes the
padded work C1+C2 hug the mean load instead of paying the max expert count
on every core.

All device operands are bf16 (f32 PSUM accumulation; ~2e-3 rel err vs the
2e-2 gate) which halves HBM traffic, and the host packs bias+W1+x+W2 into
ONE dram blob laid out in consumption order, so the input stream is a few
large DMAs (each DMA costs ~632ns of serialized descriptor-generation
regardless of size) whose per-k (w1, x) slabs carry a single semaphore --
the k-outer first wave never needs two waits per matmul.

Schedule: memset-fed seed matmuls start the PE p-state clock ramp under the
DMA window (idle > ~3.4us resets the ramp; all PE gaps are kept well under
it), a 7-bank k-outer wave paces slot A against the per-k slabs, the rest
runs k-inner, slot B's 6 single-tile groups run k-outer, and W2 streams
under phase A.  Phase-B outputs go PSUM -> bf16 SBUF on the DVE and out
via per-(slot, m) DMAs from the ACT engine, smallest slot last to shorten
the drain tail.

Toolchain note: this walrus build accepts at most ONE sync-wait per
instruction.  "Observer" ops let each engine see a DMA semaphore once
(Tile's per-engine clock elides the waits elsewhere), and a TileContext
subclass splits the final drain's waits.
"""

import sys

sys.path.insert(0, "/opt/trn_rl_repo")

import numpy as np
import ml_dtypes

import concourse.bass as bass
import concourse.mybir as mybir
import concourse.tile as tile
from concourse import bass_utils
from concourse.vector_clock import ScopedClock

B, T, E, H, NEXP = 2, 1024, 768, 3072, 4
NCORES = 8
HQ = H // 4          # hidden quarter per (core, slot): 768
KE = E // 128        # 6   k-chunks over E
KQ = HQ // 128       # 6   m/k-chunks over a hidden quarter
NSLOT = 2

BF16 = ml_dtypes.bfloat16

_MAXW = 1  # walrus allows a single sync-wait per instruction


class _SplitDrainTC(tile.TileContext):
    """TileContext whose final drain splits its sem waits across single-wait
    sync-engine event-sem instructions."""

    #: sem (ant_name) expected to fire last -- ordered to the end of the
    #: split wait chain so every earlier wait retires in its shadow.
    drain_last_sem: str | None = None

    def _drain_and_barrier(self, tick_clock, wait_clock):
        carrier = self.nc.sync.nop(nofuse=True)
        wait_clock.add_sem_waits(
            carrier.ins, ScopedClock({None: tick_clock.global_clock})
        )
        waits = list(carrier.ins.sync_info.on_wait or [])
        last = self.drain_last_sem or ""
        waits.sort(
            key=lambda w: (
                w.ant_name.startswith("DMAHW"),
                w.ant_name.startswith(last) if last else False,
            )
        )
        if len(waits) > _MAXW:
            handles = {h.name: h for h in self.sems.allocated().values()}
            carrier.ins.sync_info.on_wait = waits[:_MAXW]
            for w in waits[_MAXW:]:
                self.nc.sync.wait_ge(handles[w.ant_name], w.wait_value)
        self.nc.sync.drain()
        self.nc.all_engine_barrier()
        popped = self.nc._tile_sem_poison_stack.pop()
        assert popped is self._sem_poison
        # The sem clear runs on the sync engine after the barrier; every other
        # engine's stream has already ended and the runtime serializes NEFF
        # executions, so the closing all-engine barrier is dead time and is
        # omitted.
        self.nc.clear_and_free_semaphores(list(self.sems.allocated().values()))


_prog_cache: dict[tuple, bass.Bass] = {}
_runner_cache: dict[tuple, object] = {}


class _Runner:
    """Compile once, execute many: replicates bass2jax.run_bass_via_pjrt but
    caches the jitted shard_map executable so repeat kernel() calls skip
    retracing, and exposes device-resident execution for timing."""

    def __init__(self, nc: bass.Bass):
        import jax
        from jax.sharding import Mesh, PartitionSpec, NamedSharding
        from jax.experimental.shard_map import shard_map
        from concourse import bass2jax

        bass2jax.install_neuronx_cc_hook()
        self.jax = jax
        partition_name = (
            nc.partition_id_tensor.name if nc.partition_id_tensor else None
        )
        in_names, out_names, out_avals, zero_outs = [], [], [], []
        for alloc in nc.m.functions[0].allocations:
            if not isinstance(alloc, mybir.MemoryLocationSet):
                continue
            name = alloc.memorylocations[0].name
            if alloc.kind == "ExternalInput":
                if name != partition_name:
                    in_names.append(name)
            elif alloc.kind == "ExternalOutput":
                shape = tuple(alloc.tensor_shape)
                dtype = mybir.dt.np(alloc.dtype)
                out_names.append(name)
                out_avals.append(jax.core.ShapedArray(shape, dtype))
                zero_outs.append(np.zeros(shape, dtype))
        self.in_names = list(in_names)
        self.out_names = out_names
        self.out_avals = out_avals
        self.zero_outs = zero_outs
        n_params = len(in_names)
        self.n_params = n_params
        all_in_names = list(in_names) + list(out_names)
        if partition_name is not None:
            all_in_names.append(partition_name)

        def _body(*args):
            operands = list(args)
            if partition_name is not None:
                operands.append(bass2jax.partition_id_tensor())
            outs = bass2jax._bass_exec_p.bind(
                *operands,
                out_avals=tuple(out_avals),
                in_names=tuple(all_in_names),
                out_names=tuple(out_names),
                lowering_input_output_aliases=(),
                sim_require_finite=True,
                sim_require_nnan=True,
                nc=nc,
            )
            return tuple(outs)

        devices = jax.devices()[:NCORES]
        self.mesh = Mesh(np.asarray(devices), ("core",))
        self.pspec = PartitionSpec("core")
        self.sharding = NamedSharding(self.mesh, self.pspec)
        n_outs = len(out_names)
        donate = tuple(range(n_params, n_params + n_outs))
        self.sharded = jax.jit(
            shard_map(
                _body,
                mesh=self.mesh,
                in_specs=(self.pspec,) * (n_params + n_outs),
                out_specs=(self.pspec,) * n_outs,
                check_rep=False,
            ),
            donate_argnums=donate,
            keep_unused=True,
        )

    def concat_inputs(self, in_maps):
        return [
            np.concatenate([np.asarray(m[name]) for m in in_maps], axis=0)
            for name in self.in_names
        ]

    def concat_zeros(self):
        return [
            np.zeros((NCORES * z.shape[0], *z.shape[1:]), z.dtype)
            for z in self.zero_outs
        ]

    def __call__(self, in_maps):
        out_arrs = self.sharded(*self.concat_inputs(in_maps), *self.concat_zeros())
        results = []
        for c in range(NCORES):
            results.append(
                {
                    name: np.asarray(out_arrs[i]).reshape(
                        NCORES, *self.out_avals[i].shape
                    )[c]
                    for i, name in enumerate(self.out_names)
                }
            )
        return results


def _get_runner(key) -> _Runner:
    r = _runner_cache.get(key)
    if r is None:
        nc = _prog_cache.get(key)
        if nc is None:
            nc = _build_program(key)
            _prog_cache[key] = nc
        r = _Runner(nc)
        _runner_cache[key] = r
    return r


def _ncol(C: int) -> int:
    return -(-C // 512)  # PSUM f32 bank holds 512 columns


def _round_cap(n: int) -> int:
    C = max(2, ((n + 1) // 2) * 2)
    while C % _ncol(C):
        C += 2
    return C


_W1HEAD = 512  # wave-1 only touches m0..m3 of the first w1 chunk


def _layout(C1: int, C2: int):
    """Column offsets (bf16 elements) into the packed input blob.  The first
    w1 chunk is split: its m0-3 head rides in the first slab (all the first
    k-block of the wave needs), the m4-5 tail in the second."""
    caps = (C1, C2)
    off = 0
    bias = off
    off += NSLOT * KQ                       # b1, one column per (slot, m)
    w1 = {}
    xg = {}
    w1t = None
    for s in range(NSLOT):
        if s == 1:
            # the (0,0) w1 tail rides at the head of slab (1,0): it is only
            # needed by the k-inner rest groups, well after slot A's wave,
            # and keeping it out of slab (0,1) evens the early slab sizes
            # the wave is paced by.
            w1t = off
            off += HQ - _W1HEAD
        for k in range(KE):
            w1[(s, k)] = off
            off += _W1HEAD if (s, k) == (0, 0) else HQ
            xg[(s, k)] = off
            off += caps[s]
    w2 = {}
    for s in range(NSLOT):
        w2[s] = off
        off += KQ * E
    return bias, w1, xg, w2, w1t, off


def _build_program(key) -> bass.Bass:
    """One SPMD program: per-core two-slot expert-quarter MLP."""
    C1, C2 = key
    caps = (C1, C2)
    f32 = mybir.dt.float32
    bf16 = mybir.dt.bfloat16
    nc = bass.Bass("TRN2", target_bir_lowering=False, num_devices=NCORES)

    bias_o, w1_o, xg_o, w2_o, w1t_o, W = _layout(C1, C2)
    blob = nc.dram_tensor("blob", [128, W], bf16, kind="ExternalInput")
    y = nc.dram_tensor("y", [E, C1 + C2], bf16, kind="ExternalOutput")

    # column tiles per slot (PSUM bank holds 512 f32)
    cols = {}
    for s in range(NSLOT):
        ncol = _ncol(caps[s])
        w = caps[s] // ncol
        cols[s] = [(n * w, w) for n in range(ncol)]

    with _SplitDrainTC(nc) as tc:
        with (
            tc.tile_pool(name="wpool", bufs=1) as wp,
            tc.tile_pool(name="ps", bufs=7, space="PSUM") as ps,
            tc.tile_pool(name="psW", bufs=1, space="PSUM") as psw,
        ):
            bs = wp.tile([128, W], bf16, tag="blob")
            hsA = wp.tile([128, KQ, C1], bf16, tag="hA")
            hsB = wp.tile([128, KQ, C2], bf16, tag="hB")
            hs = (hsA, hsB)
            ys = wp.tile([128, KQ, C1 + C2], bf16, tag="ys")
            seed = wp.tile([128, 2], bf16, tag="seed")
            seed2 = wp.tile([128, 1408], bf16, tag="seed2")
            bv = blob.ap()

            def w1ap(s, k, m):
                if (s, k) == (0, 0) and m * 128 >= _W1HEAD:
                    o = w1t_o + m * 128 - _W1HEAD
                else:
                    o = w1_o[(s, k)] + m * 128
                return bs[:, o : o + 128]

            def xap(s, k, c0, cw):
                o = xg_o[(s, k)] + c0
                return bs[:, o : o + cw]

            def w2ap(s, k, m):
                o = w2_o[s] + k * E + m * 128
                return bs[:, o : o + 128]

            def biasap(s, m):
                o = bias_o + s * KQ + m
                return bs[:, o : o + 1]

            # PE p-state seeds: the clock ramp is keyed to the start of the
            # PE's continuous-busy run (idle > ~3.4us resets it).  A matmul
            # on a small memset tile at ~0.5us starts the ramp; a second one
            # gated on a deliberately long DVE memset lands ~2us to bridge
            # the gap until the first slab's semaphore fires.
            nc.vector.memset(seed[:], 0.0)
            warm = psw.tile([2, 40], f32, tag="warm")
            obs_i = [0]

            def pe_obs(src):
                i = obs_i[0]
                obs_i[0] += 1
                nc.tensor.matmul(
                    warm[:, 2 * i : 2 * i + 2], src, src, start=True, stop=True
                )

            pe_obs(seed[:, 0:2])
            nc.vector.memset(seed2[:], 0.0)
            pe_obs(seed2[:, 0:2])

            # Input DMAs: consumption-order slabs of the blob.  Slab (s, k)
            # carries bias (k=0, slot A only) + w1 chunk + x chunk in ONE
            # transfer -> one semaphore per k-block of the wave.  The first
            # slab is trimmed to exactly the first k-block's needs (bias,
            # w1 m0-3, x); the (0,0) w1 tail rides with slab (0,1).  W2
            # halves stream under phase A.
            for s in range(NSLOT):
                for k in range(KE):
                    if (s, k) == (0, 0):
                        a = bias_o
                    elif (s, k) == (1, 0):
                        a = w1t_o
                    else:
                        a = w1_o[(s, k)]
                    b = xg_o[(s, k)] + caps[s]
                    nc.sync.dma_start(out=bs[:, a:b], in_=bv[:, a:b])
            for s in range(NSLOT):
                a = w2_o[s]
                nc.sync.dma_start(out=bs[:, a : a + KQ * E], in_=bv[:, a : a + KQ * E])

            # ACT observer of slab (0,0) so Gelu's bias read needs no wait.
            scratch = wp.tile([128, 12], f32, tag="actwarm")
            nc.scalar.activation(
                scratch[:, 0:1], bs[:, bias_o : bias_o + 1],
                mybir.ActivationFunctionType.Copy,
            )

            # Phase A: h_s = gelu(x_s @ W1_s + b1_s), feature-major [HQ, C_s].
            # Slot A: first 7 (m, n) groups k-outer across 7 PSUM banks,
            # paced by the per-k slabs (first matmul of each k-block carries
            # that slab's single sem), rest k-inner.  Slot B follows.
            for s in range(NSLOT):
                if s == 1:
                    # let the PE see slab (1,0) before slot B's first wave
                    # matmul, whose only remaining wait is then the PSUM
                    # recycle sem.
                    pe_obs(bs[:, w1t_o : w1t_o + 2])
                groups = [(m, n) for m in range(KQ) for n in range(len(cols[s]))]
                wave = groups[:7]
                rest = groups[7:]
                accs = {}
                for g in wave:
                    acc = ps.tile([128, cols[s][g[1]][1]], f32, tag="acc")
                    accs[g] = acc
                for k in range(KE):
                    for m, n in wave:
                        c0, cw = cols[s][n]
                        nc.tensor.matmul(
                            accs[(m, n)][:],
                            w1ap(s, k, m),
                            xap(s, k, c0, cw),
                            start=(k == 0),
                            stop=(k == KE - 1),
                        )
                for m, n in wave:
                    c0, cw = cols[s][n]
                    nc.scalar.activation(
                        hs[s][:, m, c0 : c0 + cw],
                        accs[(m, n)][:],
                        mybir.ActivationFunctionType.Gelu,
                        bias=biasap(s, m),
                    )
                for m, n in rest:
                    c0, cw = cols[s][n]
                    acc = ps.tile([128, cw], f32, tag="acc")
                    for k in range(KE):
                        nc.tensor.matmul(
                            acc[:],
                            w1ap(s, k, m),
                            xap(s, k, c0, cw),
                            start=(k == 0),
                            stop=(k == KE - 1),
                        )
                    nc.scalar.activation(
                        hs[s][:, m, c0 : c0 + cw],
                        acc[:],
                        mybir.ActivationFunctionType.Gelu,
                        bias=biasap(s, m),
                    )

            # w2 observers between the phases (their DMAs streamed under
            # phase A); ACT observers of the six slot-B slabs and both w2
            # slabs cover the HWDGE lane-FIFO sems the eight output DMAs
            # will reuse (outputs land on lanes 6,7,0..5 whose predecessors
            # are exactly those inputs; no output lane is reused).
            pe_obs(bs[:, w2_o[0] : w2_o[0] + 2])
            pe_obs(bs[:, w2_o[1] : w2_o[1] + 2])
            for i in range(KE):
                nc.scalar.activation(
                    scratch[:, i + 1 : i + 2],
                    bs[:, w1_o[(1, i)] : w1_o[(1, i)] + 1],
                    mybir.ActivationFunctionType.Copy,
                )
            for j in range(NSLOT):
                nc.scalar.activation(
                    scratch[:, KE + 1 + j : KE + 2 + j],
                    bs[:, w2_o[j] + 2 : w2_o[j] + 3],
                    mybir.ActivationFunctionType.Copy,
                )

            # Phase B: y_s = h_s @ W2_s, feature-major [E, C_s]; PSUM -> bf16
            # SBUF on the DVE; m-outer over both slots so one output DMA per
            # m covers slot A and slot B columns together -- except the last
            # m, which runs slot B first and slot A per tile, each piece
            # DMA'd as soon as it is copied, so the final DMA (and the drain
            # tail behind it) is a single narrow slot-A tile.
            yv = y.ap().rearrange("(m p) c -> p m c", p=128)
            ycol = {0: 0, 1: C1}
            split_a = len(cols[0]) <= 2  # keep total output DMAs <= 8 lanes
            for m in range(KE):
                final_m = m == KE - 1
                slot_order = [1, 0] if final_m else [0, 1]
                for s in slot_order:
                    for c0, cw in cols[s]:
                        acc = ps.tile([128, cw], f32, tag="acc")
                        for k in range(KQ):
                            nc.tensor.matmul(
                                acc[:],
                                w2ap(s, k, m),
                                hs[s][:, k, c0 : c0 + cw],
                                start=(k == 0),
                                stop=(k == KQ - 1),
                            )
                        o = ycol[s] + c0
                        nc.vector.tensor_copy(ys[:, m, o : o + cw], acc[:])
                        if final_m and s == 0 and split_a:
                            nc.scalar.dma_start(
                                out=yv[:, m, o : o + cw],
                                in_=ys[:, m, o : o + cw],
                            )
                    if final_m and (s == 1 or not split_a):
                        nc.scalar.dma_start(
                            out=yv[:, m, ycol[s] : ycol[s] + caps[s]],
                            in_=ys[:, m, ycol[s] : ycol[s] + caps[s]],
                        )
                if not final_m:
                    nc.scalar.dma_start(out=yv[:, m, :], in_=ys[:, m, :])

            # the drain's gating sem is the final output DMA's lane
            n_dmas = NSLOT * KE + NSLOT + KE + (len(cols[0]) if split_a else 1) + 1
            tc.drain_last_sem = f"DMAHW{(n_dmas - 1) % 8}"

    return nc


def kernel(x, Wg, bg, W1, b1, W2, b2):
    x = np.asarray(x, dtype=np.float32)
    Wg = np.asarray(Wg, dtype=np.float32)
    bg = np.asarray(bg, dtype=np.float32)
    W1 = np.asarray(W1, dtype=np.float32)
    b1 = np.asarray(b1, dtype=np.float32)
    W2 = np.asarray(W2, dtype=np.float32)
    b2 = np.asarray(b2, dtype=np.float32)

    x2d = x.reshape(-1, E)  # [B*T, E]
    ntok = x2d.shape[0]

    # --- dispatch (host): gate + top-1 routing, gather per-expert tokens ---
    logits = x2d @ Wg + bg
    top = np.argmax(logits, axis=-1)
    idx = [np.nonzero(top == e)[0] for e in range(NEXP)]
    counts = np.array([len(i) for i in idx])
    order = np.argsort(-counts, kind="stable")  # heavy -> light
    # slot A serves the two heaviest experts (order[0] on cores 0-3,
    # order[1] on cores 4-7), slot B the two lightest.
    slot_exp = [(order[0], order[1]), (order[2], order[3])]
    C1 = int(_round_cap(max(1, counts[slot_exp[0][0]], counts[slot_exp[0][1]])))
    C2 = int(_round_cap(max(1, counts[slot_exp[1][0]], counts[slot_exp[1][1]])))
    caps = (C1, C2)

    bias_o, w1_o, xg_o, w2_o, w1t_o, W = _layout(C1, C2)
    run = _get_runner((C1, C2))

    x16 = [x2d[idx[e]].astype(BF16).T for e in range(NEXP)]  # [E, n_e]
    W1b = W1.astype(BF16)
    W2b = W2.astype(BF16)
    b1b = b1.astype(BF16)

    in_maps = []
    for c in range(NCORES):
        q = c % 4
        blob = np.zeros((128, W), dtype=BF16)
        for s in range(NSLOT):
            e = int(slot_exp[s][c // 4])
            hq = slice(q * HQ, (q + 1) * HQ)
            # b1 quarter -> one column per m-chunk: [128, KQ] layout has
            # column (s*KQ + m) holding b1[e][q*HQ + m*128 : ... + 128]
            bq = b1b[e][hq].reshape(KQ, 128).T
            blob[:, bias_o + s * KQ : bias_o + (s + 1) * KQ] = bq
            w1q = W1b[e][:, hq]                       # [E, HQ]
            xe = x16[e]
            for k in range(KE):
                chunk = w1q[k * 128 : (k + 1) * 128]
                if (s, k) == (0, 0):
                    blob[:, w1_o[(s, k)] : w1_o[(s, k)] + _W1HEAD] = chunk[
                        :, :_W1HEAD
                    ]
                    blob[:, w1t_o : w1t_o + HQ - _W1HEAD] = chunk[:, _W1HEAD:]
                else:
                    blob[:, w1_o[(s, k)] : w1_o[(s, k)] + HQ] = chunk
                blob[:, xg_o[(s, k)] : xg_o[(s, k)] + xe.shape[1]] = xe[
                    k * 128 : (k + 1) * 128
                ]
            w2q = W2b[e][hq]                          # [HQ, E]
            for k in range(KQ):
                blob[
                    :, w2_o[s] + k * E : w2_o[s] + (k + 1) * E
                ] = w2q[k * 128 : (k + 1) * 128]
        in_maps.append({"blob": blob})
    results = run(in_maps)

    # --- combine (host): sum quarter-partials per expert, + b2, scatter ---
    out = np.zeros((ntok, E), dtype=np.float32)
    for s in range(NSLOT):
        c0 = 0 if s == 0 else caps[0]
        for half, e in enumerate(slot_exp[s]):
            e = int(e)
            n = len(idx[e])
            if n == 0:
                continue
            acc = np.zeros((E, n), dtype=np.float32)
            for c in range(4 * half, 4 * half + 4):
                ye = results[c]["y"][:, c0 : c0 + caps[s]]
                acc += ye[:, :n].astype(np.float32)
            out[idx[e]] = acc.T + b2[e]
    return out.reshape(B, T, E)
